# revision 14
# baseline (speedup 1.0000x reference)
import sys

sys.path.insert(0, '/opt/trn_rl_repo')

import numpy as np
import ml_dtypes
import concourse.bass as bass
from concourse import bacc
import concourse.mybir as mybir
import concourse.tile as tile
from concourse.masks import make_identity

S = 4096
H = 1024
I_ = 4096
NH = 16
HD = 64
NC = 8
SM = S // NC
DM = 128
HC = H // 128
ST = S // 512
NT = S // 128
F32 = mybir.dt.float32
F32R = mybir.dt.float32r
BF16 = mybir.dt.bfloat16
AF = mybir.ActivationFunctionType
ALU = mybir.AluOpType

_CACHE = {}


def build_nc():
    nc = bacc.Bacc(None, target_bir_lowering=False, debug=False)

    def P(name, shape, dt=F32):
        return nc.declare_dram_parameter(name, shape, dt, isOutput=False)

    x_p = P("x", [S, H])
    x_own = P("x_own", [SM, H])
    wqkvT = P("wqkvT", [H, 3 * DM], BF16)
    bqkv = P("bqkv", [1, 3 * DM])
    owT = P("owT", [DM, H], BF16)
    ob = P("ob", [1, H])
    ff1w3 = P("ff1w3", [32, 128, H], BF16)
    ff1b = P("ff1b", [32, 128])
    ff2wT = P("ff2wT", [I_, H], BF16)
    ffb2 = P("ffb2", [1, H])
    y = nc.declare_dram_parameter("y", [SM, H], F32, isOutput=True)

    with tile.TileContext(nc) as tc:
        cst = tc.alloc_tile_pool(name="cst", bufs=1)
        dram = tc.alloc_tile_pool(name="dram", bufs=1, space="DRAM")
        setp = tc.alloc_tile_pool(name="setp", bufs=1)
        ps_set = tc.alloc_tile_pool(name="ps_set", bufs=2, space="PSUM")

        rs_in = dram.tile([S, H], BF16)
        rs_out = dram.tile([SM, H], BF16)

        ones_f = cst.tile([1, 128], F32)
        nc.gpsimd.memset(ones_f, 1.0)
        ones_r = cst.tile([1, 128], F32R)
        nc.vector.tensor_copy(ones_r[:], ones_f[:])

        def load_vec(p):
            t = setp.tile([1, H], F32, tag=f"v_{p.name}")
            nc.sync.dma_start(out=t[:], in_=p[:])
            return t

        ob_v, ffb2_v = load_vec(ob), load_vec(ffb2)

        def bcast(v, name):
            bc = cst.tile([128, H], F32, tag=f"bc_{name}", name=f"bc_{name}")
            for hf in range(H // 512):
                ps = ps_set.tile([128, 512], F32)
                nc.tensor.matmul(ps[:], ones_f[0:1, :], v[0:1, hf * 512:(hf + 1) * 512],
                                 start=True, stop=True)
                nc.vector.tensor_copy(bc[:, hf * 512:(hf + 1) * 512], ps[:])
            return bc

        ob_bc = bcast(ob_v, "ob")
        ffb2_bc = bcast(ffb2_v, "ffb2")

        bqkv_sb = setp.tile([1, 3 * DM], F32)
        nc.sync.dma_start(out=bqkv_sb[:], in_=bqkv[:])
        qkvb_pp = []
        for j in range(3):
            ps = ps_set.tile([128, 512], F32)
            nc.tensor.matmul(ps[:, 0:1],
                             bqkv_sb[0:1, j * 128:(j + 1) * 128],
                             ones_f[0:1, 0:1], start=True, stop=True)
            t = cst.tile([128, 1], F32, tag=f"b_pp{j}")
            nc.vector.tensor_copy(t[:], ps[:, 0:1])
            qkvb_pp.append(t)

        ident_b = cst.tile([128, 128], BF16)
        make_identity(nc, ident_b)
        ident_f = cst.tile([32, 32], F32)
        make_identity(nc, ident_f)
        ffb1_ld = setp.tile([32, 128], F32)
        nc.sync.dma_start(out=ffb1_ld[:], in_=ff1b[:])
        ps = ps_set.tile([128, 512], F32)
        nc.tensor.transpose(ps[:, 0:32], ffb1_ld[:], ident_f[:])
        ffb1_pp = cst.tile([128, 32], F32)
        nc.vector.tensor_copy(ffb1_pp[:], ps[:, 0:32])

        wq_sb = []
        for hc in range(HC):
            t = cst.tile([128, 3 * DM], BF16, tag=f"wqkv{hc}")
            nc.sync.dma_start(out=t[:], in_=wqkvT[hc * 128:(hc + 1) * 128, :])
            wq_sb.append(t)
        owT_sb = cst.tile([DM, H], BF16, tag="owT")
        nc.sync.dma_start(out=owT_sb[:], in_=owT[:])

        ps_set.release()
        setp.release()

        ff = tc.alloc_tile_pool(name="ff", bufs=1)
        attnp = tc.alloc_tile_pool(name="attnp", bufs=1)
        xntp = tc.alloc_tile_pool(name="xntp", bufs=1)

        xnT = xntp.tile([128, HC, S], BF16, name="xnT")
        with tc.tile_pool(name="xp", bufs=6) as xp, \
             tc.tile_pool(name="xcp", bufs=8) as xcp, \
             tc.tile_pool(name="lns", bufs=8) as lns:
            LAG = 6
            xn_q = []
            for t in range(NT + LAG):
                if t < NT:
                    xt = xp.tile([128, H], F32, tag="x")
                    nc.scalar.dma_start(out=xt[:], in_=x_p[t * 128:(t + 1) * 128, :])
                    xc = xcp.tile([128, H], BF16, tag="xc")
                    sums = lns.tile([128, 1], F32, tag="sums")
                    nc.scalar.activation(xc[:], xt[:], AF.Copy, accum_out=sums[:])
                    nm = lns.tile([128, 1], F32, tag="nm")
                    nc.scalar.mul(nm[:], sums[:], -1.0 / H)
                    xn = xcp.tile([128, H], BF16, tag="xn")
                    nc.vector.tensor_scalar_add(xn[:], xc[:], nm[:])
                    xn_q.append(xn)
                if t >= LAG:
                    tp = t - LAG
                    nc.sync.dma_start_transpose(
                        out=xnT[:, :, tp * 128:(tp + 1) * 128], in_=xn_q[tp][:])

        QTp = attnp.tile([128, S], BF16, name="QTp")
        KTp = attnp.tile([128, S], BF16, name="KTp")
        vb = attnp.tile([128, NT, 130], BF16, name="vb")
        nc.gpsimd.memset(vb[:, :, 64:65], 1.0)
        nc.gpsimd.memset(vb[:, :, 129:130], 1.0)

        with tc.tile_pool(name="ps_qkv", bufs=2, space="PSUM") as ps_qkv, \
             tc.tile_pool(name="vt_sb", bufs=3) as vt_sb:
            def v_trans(r, vt):
                for tb in range(4):
                    ti = r * 4 + tb
                    vtp = ps_qkv.tile([128, 128], BF16, tag="vtp")
                    nc.tensor.transpose(vtp[:], vt[:, tb * 128:(tb + 1) * 128],
                                        ident_b[:])
                    for h in range(2):
                        nc.vector.tensor_copy(vb[:, ti, h * 65:h * 65 + 64],
                                              vtp[:, h * 64:(h + 1) * 64])

            vt_prev = None
            for r in range(ST):
                sl = slice(r * 512, (r + 1) * 512)
                qk = ps_qkv.tile([128, 1024], F32, tag="qk")
                for j, dest in ((0, QTp), (1, KTp)):
                    for hc in range(HC):
                        nc.tensor.matmul(qk[:, j * 512:(j + 1) * 512],
                                         wq_sb[hc][:, j * 128:(j + 1) * 128],
                                         xnT[:, hc, sl],
                                         start=(hc == 0), stop=(hc == 7))
                    nc.vector.tensor_scalar_add(
                        dest[:, sl], qk[:, j * 512:(j + 1) * 512], qkvb_pp[j][:])
                if vt_prev is not None:
                    v_trans(r - 1, vt_prev)
                vps = ps_qkv.tile([128, 512], F32, tag="v")
                for hc in range(HC):
                    nc.tensor.matmul(vps[:],
                                     wq_sb[hc][:, 2 * 128:3 * 128],
                                     xnT[:, hc, sl],
                                     start=(hc == 0), stop=(hc == 7))
                vt = vt_sb.tile([128, 512], BF16, tag="vt")
                nc.vector.tensor_scalar_add(vt[:], vps[:], qkvb_pp[2][:])
                vt_prev = vt
            v_trans(ST - 1, vt_prev)

        xntp.release()

        xn2T = ff.tile([128, HC, SM], BF16, name="xn2T")
        x2_t = []

        with tc.tile_pool(name="ps_sps", bufs=2, space="PSUM") as ps_sps, \
             tc.tile_pool(name="ps_ctx", bufs=2, space="PSUM") as ps_ctx, \
             tc.tile_pool(name="exp", bufs=3) as expp, \
             tc.tile_pool(name="ctxp", bufs=3) as ctxp, \
             tc.tile_pool(name="aop", bufs=2) as aop, \
             tc.tile_pool(name="rcp", bufs=8) as rcp, \
             tc.tile_pool(name="ffs", bufs=2) as ffs:

            def chunk_ln2(r):
                rl = ffs.tile([64, H], BF16, tag="rl")
                nc.sync.dma_start(out=rl[:], in_=rs_out[r * 64:(r + 1) * 64, :])
                rf = ffs.tile([64, H], F32, tag="rf")
                nc.vector.tensor_copy(rf[:], rl[:])
                xo = ffs.tile([64, H], F32, tag="xo")
                nc.sync.dma_start(out=xo[:], in_=x_own[r * 64:(r + 1) * 64, :])
                x2 = ff.tile([64, H], F32, tag=f"x2_{r}", name=f"x2_{r}")
                nc.vector.tensor_add(x2[:], rf[:], xo[:])
                nc.vector.tensor_add(x2[:], x2[:], ob_bc[0:64, :])
                x2_t.append(x2)
                x2c = ffs.tile([64, H], BF16, tag="x2c")
                sums = ffs.tile([64, 1], F32, tag="s2")
                nc.scalar.activation(x2c[:], x2[:], AF.Copy, accum_out=sums[:])
                nm = ffs.tile([64, 1], F32, tag="nm2")
                nc.scalar.mul(nm[:], sums[:], -1.0 / H)
                xn2 = ffs.tile([64, H], BF16, tag="xn2")
                nc.vector.tensor_scalar_add(xn2[:], x2c[:], nm[:])
                nc.sync.dma_start_transpose(
                    out=xn2T[:, :, r * 64:(r + 1) * 64], in_=xn2[:])

            cps_by_r = {}
            ex_q = {}

            def boundary(r):
                cps = cps_by_r.pop(r)
                ctxT = ctxp.tile([128, 512], BF16, tag="ctxT", name=f"ctxT{r}")
                for h in range(2):
                    rc = rcp.tile([1, 512], F32R, tag="rc")
                    with nc.allow_low_precision(reason="f32r softmax denom"):
                        nc.vector.reciprocal(rc[:], cps[h][64:65, :])
                    bps = ps_sps.tile([128, 1024], F32, tag="sps",
                                      name=f"bps{r}_{h}")
                    nc.tensor.matmul(bps[0:64, 0:512], ones_r[0:1, 0:64],
                                     rc[0:1, :], start=True, stop=True)
                    bsb = rcp.tile([64, 512], F32, tag="bsb")
                    nc.vector.tensor_copy(bsb[:], bps[0:64, 0:512])
                    nc.vector.tensor_mul(ctxT[h * 64:(h + 1) * 64, :],
                                         cps[h][0:64, :], bsb[:])
                for j in range(4):
                    ops = ps_sps.tile([128, 1024], F32, tag="sps",
                                      name=f"ops{r}_{j}")
                    ao = aop.tile([128, H], BF16, tag="ao")
                    for hf in range(2):
                        nc.tensor.matmul(ops[:, hf * 512:(hf + 1) * 512],
                                         ctxT[:, j * 128:(j + 1) * 128],
                                         owT_sb[:, hf * 512:(hf + 1) * 512],
                                         start=True, stop=True)
                        nc.vector.tensor_copy(ao[:, hf * 512:(hf + 1) * 512],
                                              ops[:, hf * 512:(hf + 1) * 512])
                    nc.sync.dma_start(
                        out=rs_in[r * 512 + j * 128:r * 512 + (j + 1) * 128, :],
                        in_=ao[:])
                nc.gpsimd.collective_compute(
                    "ReduceScatter", ALU.add, replica_groups=[list(range(NC))],
                    ins=[rs_in[r * 512:(r + 1) * 512, :]],
                    outs=[rs_out[r * 64:(r + 1) * 64, :]])
                chunk_ln2(r)

            NG = ST * NT
            for g in range(NG + 1):
                if g < NG:
                    r, t = divmod(g, NT)
                    if t == 0:
                        cps_by_r[r] = [ps_ctx.tile([65, 512], F32, tag=f"cps{h}",
                                                   name=f"cps{r}_{h}")
                                       for h in range(2)]
                    sl = slice(r * 512, (r + 1) * 512)
                    sps = ps_sps.tile([128, 1024], F32, tag="sps")
                    for h in range(2):
                        hs = slice(h * 64, (h + 1) * 64)
                        nc.tensor.matmul(sps[:, h * 512:(h + 1) * 512],
                                         KTp[hs, t * 128:(t + 1) * 128],
                                         QTp[hs, sl], start=True, stop=True)
                    ex = expp.tile([128, 1024], BF16, tag="ex")
                    nc.scalar.activation(ex[:], sps[:], AF.Exp, scale=0.125)
                    ex_q[g] = ex
                if g >= 1:
                    gp = g - 1
                    rp, tp = divmod(gp, NT)
                    for h in range(2):
                        nc.tensor.matmul(cps_by_r[rp][h][:],
                                         vb[:, tp, h * 65:(h + 1) * 65],
                                         ex_q[gp][:, h * 512:(h + 1) * 512],
                                         start=(tp == 0), stop=(tp == NT - 1),
                                         skip_group_check=True)
                    del ex_q[gp]
                if g >= NT + 2 and (g - 2) % NT == 0:
                    boundary((g - 2) // NT - 1)
            boundary(ST - 1)

        attnp.release()

        ffl = tc.alloc_tile_pool(name="ffl", bufs=1)
        with tc.tile_pool(name="w1p", bufs=8) as w1p, \
             tc.tile_pool(name="w2p", bufs=8) as w2p, \
             tc.tile_pool(name="yp", bufs=3) as yp, \
             tc.tile_pool(name="ps_f1", bufs=2, space="PSUM") as ps_f1, \
             tc.tile_pool(name="ps_f2", bufs=1, space="PSUM") as ps_f2:

            hT = [ffl.tile([128, SM], BF16, tag=f"hT{i}", name=f"hT{i}")
                  for i in range(32)]
            for it in range(32):
                w1t = w1p.tile([128, HC, 128], BF16, tag="w1")
                nc.sync.dma_start(out=w1t[:], in_=ff1w3[it, :, :])
                ps1 = ps_f1.tile([128, 512], F32, tag="f1")
                for hc in range(HC):
                    nc.tensor.matmul(ps1[:], w1t[:, hc, :], xn2T[:, hc, :],
                                     start=(hc == 0), stop=(hc == 7))
                nc.scalar.activation(hT[it][:], ps1[:], AF.Relu,
                                     bias=ffb1_pp[:, it:it + 1])

            for hf in range(2):
                sl = slice(hf * 512, (hf + 1) * 512)
                yps = [ps_f2.tile([128, 512], F32, name=f"yps{hf}_{i}",
                                  tag=f"yps{i}", bufs=1) for i in range(4)]
                for ic in range(32):
                    w2t = w2p.tile([128, 512], BF16, tag="w2")
                    nc.sync.dma_start(
                        out=w2t[:],
                        in_=ff2wT[ic * 128:(ic + 1) * 128,
                                  hf * 512:(hf + 1) * 512])
                    for s4 in range(4):
                        nc.tensor.matmul(yps[s4][:],
                                         hT[ic][:, s4 * 128:(s4 + 1) * 128],
                                         w2t[:], start=(ic == 0), stop=(ic == 31),
                                         skip_group_check=True)
                for s4 in range(4):
                    for half in range(2):
                        r = s4 * 2 + half
                        hsl = slice(half * 64, (half + 1) * 64)
                        yt = yp.tile([64, 512], F32, tag="yt")
                        nc.vector.tensor_add(yt[:], yps[s4][hsl, :],
                                             ffb2_bc[0:64, sl])
                        nc.vector.tensor_add(yt[:], yt[:], x2_t[r][:, sl])
                        nc.sync.dma_start(
                            out=y[s4 * 128 + half * 64:s4 * 128 + (half + 1) * 64,
                                  sl],
                            in_=yt[:])

        ffl.release()
        ff.release()
        dram.release()
        cst.release()

    nc.compile()
    return nc


def make_in_maps(inputs):
    f32 = lambda a: np.ascontiguousarray(np.asarray(a, dtype=np.float32))
    bf = lambda a: np.ascontiguousarray(np.asarray(a, dtype=np.float32)
                                        .astype(ml_dtypes.bfloat16))
    x = f32(inputs["x"])
    anw, anb = f32(inputs["an_w"]), f32(inputs["an_b"])
    fnw, fnb = f32(inputs["fn_w"]), f32(inputs["fn_b"])
    q_w, k_w, v_w = f32(inputs["q_w"]), f32(inputs["k_w"]), f32(inputs["v_w"])
    o_w = f32(inputs["o_w"])
    ff1_w, ff2_w = f32(inputs["ff1_w"]), f32(inputs["ff2_w"])

    w1_eff = ff1_w * fnw[None, :]
    b1_eff = f32(inputs["ff1_b"]) + ff1_w @ fnb
    ff1w3 = np.ascontiguousarray(
        w1_eff.reshape(32, 128, HC, 128).transpose(0, 3, 2, 1)
        .reshape(32, 128, H).astype(ml_dtypes.bfloat16))
    ff2wT = bf(ff2_w.T)
    ff1b = np.ascontiguousarray(b1_eff.reshape(32, 128))
    row = lambda a: np.ascontiguousarray(a.reshape(1, -1))

    in_maps = []
    for m in range(NC):
        dm = slice(m * DM, (m + 1) * DM)
        wq = (q_w[dm] * anw[None, :]).T
        wk = (k_w[dm] * anw[None, :]).T
        wv = (v_w[dm] * anw[None, :]).T
        wqkvT = np.ascontiguousarray(
            np.concatenate([wq, wk, wv], axis=1).astype(ml_dtypes.bfloat16))
        bq = f32(inputs["q_b"])[dm] + q_w[dm] @ anb
        bk = f32(inputs["k_b"])[dm] + k_w[dm] @ anb
        bv = f32(inputs["v_b"])[dm] + v_w[dm] @ anb
        in_maps.append({
            "x": x,
            "x_own": np.ascontiguousarray(x[row_perm(m)]),
            "wqkvT": wqkvT,
            "bqkv": row(np.concatenate([bq, bk, bv])),
            "owT": bf(o_w[:, dm].T),
            "ob": row(f32(inputs["o_b"])),
            "ff1w3": ff1w3,
            "ff1b": ff1b,
            "ff2wT": ff2wT,
            "ffb2": row(f32(inputs["ff2_b"])),
        })
    return in_maps


def row_perm(m):
    return np.concatenate(
        [np.arange(r * 512 + m * 64, r * 512 + (m + 1) * 64) for r in range(ST)])


def kernel(**inputs) -> np.ndarray:
    from concourse.bass_utils import run_bass_kernel_spmd
    if "nc" not in _CACHE:
        _CACHE["nc"] = build_nc()
    nc = _CACHE["nc"]
    in_maps = make_in_maps(inputs)
    res = run_bass_kernel_spmd(nc, in_maps, core_ids=list(range(NC)))
    out = np.empty((S, H), dtype=np.float32)
    for m in range(NC):
        out[row_perm(m)] = res.results[m]["y"]
    return out


# revision 17
# speedup vs baseline: 1.0690x; 1.0690x over previous
import sys

sys.path.insert(0, '/opt/trn_rl_repo')

import numpy as np
import ml_dtypes
import concourse.bass as bass
from concourse import bacc
import concourse.mybir as mybir
import concourse.tile as tile
from concourse.masks import make_identity

S = 4096
H = 1024
I_ = 4096
NH = 16
HD = 64
NC = 8
SM = S // NC
DM = 128
HC = H // 128
ST = S // 512
NT = S // 128
F32 = mybir.dt.float32
F32R = mybir.dt.float32r
BF16 = mybir.dt.bfloat16
AF = mybir.ActivationFunctionType
ALU = mybir.AluOpType
AXX = mybir.AxisListType.X

_CACHE = {}


def build_nc():
    nc = bacc.Bacc(None, target_bir_lowering=False, debug=False)

    def P(name, shape, dt=F32):
        return nc.declare_dram_parameter(name, shape, dt, isOutput=False)

    x_p = P("x", [S, H])
    x_own = P("x_own", [SM, H])
    wqkvT = P("wqkvT", [H, 3 * DM], BF16)
    bqkv = P("bqkv", [1, 3 * DM])
    owT = P("owT", [DM, H], BF16)
    ob = P("ob", [1, H])
    ff1w3 = P("ff1w3", [32, 128, H], BF16)
    ff1b = P("ff1b", [32, 128])
    ff2wT = P("ff2wT", [I_, H], BF16)
    ffb2 = P("ffb2", [1, H])
    y = nc.declare_dram_parameter("y", [SM, H], F32, isOutput=True)

    with tile.TileContext(nc) as tc:
        cst = tc.alloc_tile_pool(name="cst", bufs=1)
        dram = tc.alloc_tile_pool(name="dram", bufs=1, space="DRAM")
        setp = tc.alloc_tile_pool(name="setp", bufs=1)
        ps_set = tc.alloc_tile_pool(name="ps_set", bufs=2, space="PSUM")

        rs_in = dram.tile([S, H], BF16)
        rs_out = dram.tile([SM, H], BF16)

        ones_f = cst.tile([1, 128], F32)
        nc.gpsimd.memset(ones_f, 1.0)
        ones_b = cst.tile([1, 128], BF16)
        nc.vector.tensor_copy(ones_b[:], ones_f[:])

        def load_vec(p):
            t = setp.tile([1, H], F32, tag=f"v_{p.name}")
            nc.sync.dma_start(out=t[:], in_=p[:])
            return t

        ob_v, ffb2_v = load_vec(ob), load_vec(ffb2)

        def bcast(v, name):
            bc = cst.tile([128, H], F32, tag=f"bc_{name}", name=f"bc_{name}")
            for hf in range(H // 512):
                ps = ps_set.tile([128, 512], F32)
                nc.tensor.matmul(ps[:], ones_f[0:1, :], v[0:1, hf * 512:(hf + 1) * 512],
                                 start=True, stop=True)
                nc.vector.tensor_copy(bc[:, hf * 512:(hf + 1) * 512], ps[:])
            return bc

        ob_bc = bcast(ob_v, "ob")
        ffb2_bc = bcast(ffb2_v, "ffb2")

        bqkv_sb = setp.tile([1, 3 * DM], F32)
        nc.sync.dma_start(out=bqkv_sb[:], in_=bqkv[:])
        qkvb_pp = []
        for j in range(3):
            ps = ps_set.tile([128, 512], F32)
            nc.tensor.matmul(ps[:, 0:1],
                             bqkv_sb[0:1, j * 128:(j + 1) * 128],
                             ones_f[0:1, 0:1], start=True, stop=True)
            t = cst.tile([128, 1], F32, tag=f"b_pp{j}")
            nc.vector.tensor_copy(t[:], ps[:, 0:1])
            qkvb_pp.append(t)

        ident_b = cst.tile([128, 128], BF16)
        make_identity(nc, ident_b)
        ident_f = cst.tile([32, 32], F32)
        make_identity(nc, ident_f)
        ffb1_ld = setp.tile([32, 128], F32)
        nc.sync.dma_start(out=ffb1_ld[:], in_=ff1b[:])
        ps = ps_set.tile([128, 512], F32)
        nc.tensor.transpose(ps[:, 0:32], ffb1_ld[:], ident_f[:])
        ffb1_pp = cst.tile([128, 32], F32)
        nc.vector.tensor_copy(ffb1_pp[:], ps[:, 0:32])

        wq_sb = []
        for hc in range(HC):
            t = cst.tile([128, 3 * DM], BF16, tag=f"wqkv{hc}")
            nc.sync.dma_start(out=t[:], in_=wqkvT[hc * 128:(hc + 1) * 128, :])
            wq_sb.append(t)
        owT_sb = cst.tile([DM, H], BF16, tag="owT")
        nc.sync.dma_start(out=owT_sb[:], in_=owT[:])

        ps_set.release()
        setp.release()

        ff = tc.alloc_tile_pool(name="ff", bufs=1)
        attnp = tc.alloc_tile_pool(name="attnp", bufs=1)
        xntp = tc.alloc_tile_pool(name="xntp", bufs=1)

        xnT = xntp.tile([128, HC, S], BF16, name="xnT")
        with tc.tile_pool(name="xp", bufs=8) as xp, \
             tc.tile_pool(name="xcp", bufs=4) as xcp, \
             tc.tile_pool(name="lns", bufs=8) as lns:
            LAG = 6
            xt_q = {}
            for t in range(NT + LAG):
                if t < NT:
                    xt = xp.tile([128, H], F32, tag="x")
                    nc.sync.dma_start(out=xt[:], in_=x_p[t * 128:(t + 1) * 128, :])
                    xt_q[t] = xt
                if t >= LAG:
                    tp = t - LAG
                    xc = xcp.tile([128, H], BF16, tag="xc")
                    sums = lns.tile([128, 1], F32, tag="sums")
                    nc.scalar.activation(xc[:], xt_q.pop(tp)[:], AF.Copy,
                                         accum_out=sums[:])
                    nm = lns.tile([128, 1], F32, tag="nm")
                    nc.scalar.mul(nm[:], sums[:], -1.0 / H)
                    xn = xcp.tile([128, H], BF16, tag="xn")
                    nc.vector.tensor_scalar_add(xn[:], xc[:], nm[:])
                    nc.sync.dma_start_transpose(
                        out=xnT[:, :, tp * 128:(tp + 1) * 128], in_=xn[:])

        QTp = attnp.tile([128, S], BF16, name="QTp")
        KTp = attnp.tile([128, S], BF16, name="KTp")
        vb = attnp.tile([128, NT, 130], BF16, name="vb")
        nc.gpsimd.memset(vb[:, :, 64:65], 1.0)
        nc.gpsimd.memset(vb[:, :, 129:130], 1.0)

        with tc.tile_pool(name="ps_qkv", bufs=2, space="PSUM") as ps_qkv, \
             tc.tile_pool(name="vt_sb", bufs=3) as vt_sb:
            def v_trans(r, vt):
                for tb in range(4):
                    ti = r * 4 + tb
                    vtp = ps_qkv.tile([128, 128], BF16, tag="vtp")
                    nc.tensor.transpose(vtp[:], vt[:, tb * 128:(tb + 1) * 128],
                                        ident_b[:])
                    for h in range(2):
                        nc.vector.tensor_copy(vb[:, ti, h * 65:h * 65 + 64],
                                              vtp[:, h * 64:(h + 1) * 64])

            vt_prev = None
            for r in range(ST):
                sl = slice(r * 512, (r + 1) * 512)
                qk = ps_qkv.tile([128, 1024], F32, tag="qk")
                for j, dest in ((0, QTp), (1, KTp)):
                    for hc in range(HC):
                        nc.tensor.matmul(qk[:, j * 512:(j + 1) * 512],
                                         wq_sb[hc][:, j * 128:(j + 1) * 128],
                                         xnT[:, hc, sl],
                                         start=(hc == 0), stop=(hc == 7))
                    nc.vector.tensor_scalar_add(
                        dest[:, sl], qk[:, j * 512:(j + 1) * 512], qkvb_pp[j][:])
                if vt_prev is not None:
                    v_trans(r - 1, vt_prev)
                vps = ps_qkv.tile([128, 512], F32, tag="v")
                for hc in range(HC):
                    nc.tensor.matmul(vps[:],
                                     wq_sb[hc][:, 2 * 128:3 * 128],
                                     xnT[:, hc, sl],
                                     start=(hc == 0), stop=(hc == 7))
                vt = vt_sb.tile([128, 512], BF16, tag="vt")
                nc.vector.tensor_scalar_add(vt[:], vps[:], qkvb_pp[2][:])
                vt_prev = vt
            v_trans(ST - 1, vt_prev)

        xntp.release()

        xn2T = ff.tile([128, HC, SM], BF16, name="xn2T")
        x2_t = []

        with tc.tile_pool(name="ps_sps", bufs=2, space="PSUM") as ps_sps, \
             tc.tile_pool(name="ps_ctx", bufs=2, space="PSUM") as ps_ctx, \
             tc.tile_pool(name="exp", bufs=3) as expp, \
             tc.tile_pool(name="ctxp", bufs=3) as ctxp, \
             tc.tile_pool(name="aop", bufs=2) as aop, \
             tc.tile_pool(name="rcp", bufs=8) as rcp, \
             tc.tile_pool(name="ffs", bufs=2) as ffs:

            def chunk_ln2(r):
                rl = ffs.tile([64, H], BF16, tag="rl")
                nc.sync.dma_start(out=rl[:], in_=rs_out[r * 64:(r + 1) * 64, :])
                rf = ffs.tile([64, H], F32, tag="rf")
                nc.vector.tensor_copy(rf[:], rl[:])
                xo = ffs.tile([64, H], F32, tag="xo")
                nc.sync.dma_start(out=xo[:], in_=x_own[r * 64:(r + 1) * 64, :])
                x2 = ff.tile([64, H], F32, tag=f"x2_{r}", name=f"x2_{r}")
                nc.vector.tensor_add(x2[:], rf[:], xo[:])
                nc.vector.tensor_add(x2[:], x2[:], ob_bc[0:64, :])
                x2_t.append(x2)
                ns = ffs.tile([64, 1], F32, tag="s2")
                nc.vector.reduce_sum(out=ns[:], in_=x2[:], axis=AXX, negate=True)
                nm = ffs.tile([64, 1], F32, tag="nm2")
                nc.vector.tensor_scalar_mul(nm[:], ns[:], 1.0 / H)
                xn2 = ffs.tile([64, H], BF16, tag="xn2")
                nc.vector.tensor_scalar_add(xn2[:], x2[:], nm[:])
                nc.sync.dma_start_transpose(
                    out=xn2T[:, :, r * 64:(r + 1) * 64], in_=xn2[:])

            cps_by_r = {}
            ex_q = {}

            ctxT_by_r = {}

            def boundary_a(r):
                cps = cps_by_r.pop(r)
                ctxT = ctxp.tile([128, 512], BF16, tag="ctxT", name=f"ctxT{r}")
                for h in range(2):
                    den = rcp.tile([1, 512], F32, tag="den")
                    nc.vector.tensor_copy(den[:], cps[h][64:65, :])
                    rc = rcp.tile([1, 512], F32, tag="rc")
                    nc.vector.reciprocal_approx_fast(rc[:], den[:])
                    rcb = rcp.tile([1, 512], BF16, tag="rcb")
                    nc.vector.tensor_copy(rcb[:], rc[:])
                    bps = ps_sps.tile([128, 1024], F32, tag="sps",
                                      name=f"bps{r}_{h}")
                    nc.tensor.matmul(bps[0:64, 0:512], ones_b[0:1, 0:64],
                                     rcb[0:1, :], start=True, stop=True)
                    bsb = rcp.tile([64, 512], F32, tag="bsb")
                    nc.vector.tensor_copy(bsb[:], bps[0:64, 0:512])
                    nc.vector.tensor_mul(ctxT[h * 64:(h + 1) * 64, :],
                                         cps[h][0:64, :], bsb[:])
                ctxT_by_r[r] = ctxT

            def boundary_b(r):
                ctxT = ctxT_by_r.pop(r)
                for j in range(4):
                    ops = ps_sps.tile([128, 1024], F32, tag="sps",
                                      name=f"ops{r}_{j}")
                    ao = aop.tile([128, H], BF16, tag="ao")
                    for hf in range(2):
                        nc.tensor.matmul(ops[:, hf * 512:(hf + 1) * 512],
                                         ctxT[:, j * 128:(j + 1) * 128],
                                         owT_sb[:, hf * 512:(hf + 1) * 512],
                                         start=True, stop=True)
                        nc.vector.tensor_copy(ao[:, hf * 512:(hf + 1) * 512],
                                              ops[:, hf * 512:(hf + 1) * 512])
                    nc.sync.dma_start(
                        out=rs_in[r * 512 + j * 128:r * 512 + (j + 1) * 128, :],
                        in_=ao[:])
                nc.gpsimd.collective_compute(
                    "ReduceScatter", ALU.add, replica_groups=[list(range(NC))],
                    ins=[rs_in[r * 512:(r + 1) * 512, :]],
                    outs=[rs_out[r * 64:(r + 1) * 64, :]])
                chunk_ln2(r)

            NG = ST * NT
            for g in range(NG + 1):
                if g < NG:
                    r, t = divmod(g, NT)
                    if t == 0:
                        cps_by_r[r] = [ps_ctx.tile([65, 512], F32, tag=f"cps{h}",
                                                   name=f"cps{r}_{h}")
                                       for h in range(2)]
                    sl = slice(r * 512, (r + 1) * 512)
                    sps = ps_sps.tile([128, 1024], F32, tag="sps")
                    for h in range(2):
                        hs = slice(h * 64, (h + 1) * 64)
                        nc.tensor.matmul(sps[:, h * 512:(h + 1) * 512],
                                         KTp[hs, t * 128:(t + 1) * 128],
                                         QTp[hs, sl], start=True, stop=True)
                    ex = expp.tile([128, 1024], BF16, tag="ex")
                    nc.scalar.activation(ex[:], sps[:], AF.Exp, scale=0.125)
                    ex_q[g] = ex
                if g >= 1:
                    gp = g - 1
                    rp, tp = divmod(gp, NT)
                    for h in range(2):
                        nc.tensor.matmul(cps_by_r[rp][h][:],
                                         vb[:, tp, h * 65:(h + 1) * 65],
                                         ex_q[gp][:, h * 512:(h + 1) * 512],
                                         start=(tp == 0), stop=(tp == NT - 1),
                                         skip_group_check=True)
                    del ex_q[gp]
                if g >= NT + 2 and (g - 2) % NT == 0:
                    boundary_a((g - 2) // NT - 1)
                if g >= NT + 5 and (g - 5) % NT == 0:
                    boundary_b((g - 5) // NT - 1)
            boundary_a(ST - 1)
            boundary_b(ST - 1)

        attnp.release()

        ffl = tc.alloc_tile_pool(name="ffl", bufs=1)
        with tc.tile_pool(name="w1p", bufs=8) as w1p, \
             tc.tile_pool(name="w2p", bufs=8) as w2p, \
             tc.tile_pool(name="yp", bufs=3) as yp, \
             tc.tile_pool(name="ps_f1", bufs=2, space="PSUM") as ps_f1, \
             tc.tile_pool(name="ps_f2", bufs=1, space="PSUM") as ps_f2:

            hT = [ffl.tile([128, SM], BF16, tag=f"hT{i}", name=f"hT{i}")
                  for i in range(32)]
            for it in range(32):
                w1t = w1p.tile([128, HC, 128], BF16, tag="w1")
                nc.sync.dma_start(out=w1t[:], in_=ff1w3[it, :, :])
                ps1 = ps_f1.tile([128, 512], F32, tag="f1")
                for hc in range(HC):
                    nc.tensor.matmul(ps1[:], w1t[:, hc, :], xn2T[:, hc, :],
                                     start=(hc == 0), stop=(hc == 7))
                nc.scalar.activation(hT[it][:], ps1[:], AF.Relu,
                                     bias=ffb1_pp[:, it:it + 1])

            for hf in range(2):
                sl = slice(hf * 512, (hf + 1) * 512)
                yps = [ps_f2.tile([128, 512], F32, name=f"yps{hf}_{i}",
                                  tag=f"yps{i}", bufs=1) for i in range(4)]
                for ic in range(32):
                    w2t = w2p.tile([128, 512], BF16, tag="w2")
                    nc.sync.dma_start(
                        out=w2t[:],
                        in_=ff2wT[ic * 128:(ic + 1) * 128,
                                  hf * 512:(hf + 1) * 512])
                    for s4 in range(4):
                        nc.tensor.matmul(yps[s4][:],
                                         hT[ic][:, s4 * 128:(s4 + 1) * 128],
                                         w2t[:], start=(ic == 0), stop=(ic == 31),
                                         skip_group_check=True)
                for s4 in range(4):
                    for half in range(2):
                        r = s4 * 2 + half
                        hsl = slice(half * 64, (half + 1) * 64)
                        yt = yp.tile([64, 512], F32, tag="yt")
                        nc.vector.tensor_add(yt[:], yps[s4][hsl, :],
                                             ffb2_bc[0:64, sl])
                        nc.vector.tensor_add(yt[:], yt[:], x2_t[r][:, sl])
                        nc.sync.dma_start(
                            out=y[s4 * 128 + half * 64:s4 * 128 + (half + 1) * 64,
                                  sl],
                            in_=yt[:])

        ffl.release()
        ff.release()
        dram.release()
        cst.release()

    nc.compile()
    return nc


def make_in_maps(inputs):
    f32 = lambda a: np.ascontiguousarray(np.asarray(a, dtype=np.float32))
    bf = lambda a: np.ascontiguousarray(np.asarray(a, dtype=np.float32)
                                        .astype(ml_dtypes.bfloat16))
    x = f32(inputs["x"])
    anw, anb = f32(inputs["an_w"]), f32(inputs["an_b"])
    fnw, fnb = f32(inputs["fn_w"]), f32(inputs["fn_b"])
    q_w, k_w, v_w = f32(inputs["q_w"]), f32(inputs["k_w"]), f32(inputs["v_w"])
    o_w = f32(inputs["o_w"])
    ff1_w, ff2_w = f32(inputs["ff1_w"]), f32(inputs["ff2_w"])

    w1_eff = ff1_w * fnw[None, :]
    b1_eff = f32(inputs["ff1_b"]) + ff1_w @ fnb
    ff1w3 = np.ascontiguousarray(
        w1_eff.reshape(32, 128, HC, 128).transpose(0, 3, 2, 1)
        .reshape(32, 128, H).astype(ml_dtypes.bfloat16))
    ff2wT = bf(ff2_w.T)
    ff1b = np.ascontiguousarray(b1_eff.reshape(32, 128))
    row = lambda a: np.ascontiguousarray(a.reshape(1, -1))

    in_maps = []
    for m in range(NC):
        dm = slice(m * DM, (m + 1) * DM)
        wq = (q_w[dm] * anw[None, :]).T
        wk = (k_w[dm] * anw[None, :]).T
        wv = (v_w[dm] * anw[None, :]).T
        wqkvT = np.ascontiguousarray(
            np.concatenate([wq, wk, wv], axis=1).astype(ml_dtypes.bfloat16))
        bq = f32(inputs["q_b"])[dm] + q_w[dm] @ anb
        bk = f32(inputs["k_b"])[dm] + k_w[dm] @ anb
        bv = f32(inputs["v_b"])[dm] + v_w[dm] @ anb
        in_maps.append({
            "x": x,
            "x_own": np.ascontiguousarray(x[row_perm(m)]),
            "wqkvT": wqkvT,
            "bqkv": row(np.concatenate([bq, bk, bv])),
            "owT": bf(o_w[:, dm].T),
            "ob": row(f32(inputs["o_b"])),
            "ff1w3": ff1w3,
            "ff1b": ff1b,
            "ff2wT": ff2wT,
            "ffb2": row(f32(inputs["ff2_b"])),
        })
    return in_maps


def row_perm(m):
    return np.concatenate(
        [np.arange(r * 512 + m * 64, r * 512 + (m + 1) * 64) for r in range(ST)])


def kernel(**inputs) -> np.ndarray:
    from concourse.bass_utils import run_bass_kernel_spmd
    if "nc" not in _CACHE:
        _CACHE["nc"] = build_nc()
    nc = _CACHE["nc"]
    in_maps = make_in_maps(inputs)
    res = run_bass_kernel_spmd(nc, in_maps, core_ids=list(range(NC)))
    out = np.empty((S, H), dtype=np.float32)
    for m in range(NC):
        out[row_perm(m)] = res.results[m]["y"]
    return out


# revision 18
# speedup vs baseline: 1.2500x; 1.1693x over previous
import sys

sys.path.insert(0, '/opt/trn_rl_repo')

import numpy as np
import ml_dtypes
import concourse.bass as bass
from concourse import bacc
import concourse.mybir as mybir
import concourse.tile as tile
from concourse.masks import make_identity

S = 4096
H = 1024
I_ = 4096
NH = 16
HD = 64
NC = 8
SM = S // NC
DM = 128
HC = H // 128
ST = S // 512
NT = S // 128
F32 = mybir.dt.float32
F32R = mybir.dt.float32r
BF16 = mybir.dt.bfloat16
AF = mybir.ActivationFunctionType
ALU = mybir.AluOpType
AXX = mybir.AxisListType.X

_CACHE = {}


def build_nc():
    nc = bacc.Bacc(None, target_bir_lowering=False, debug=False)

    def P(name, shape, dt=F32):
        return nc.declare_dram_parameter(name, shape, dt, isOutput=False)

    x_bf = P("x_bf", [S, H], BF16)
    x_own = P("x_own", [SM, H])
    wqkvT = P("wqkvT", [H, 3 * DM], BF16)
    bqkv = P("bqkv", [1, 3 * DM])
    wsum3 = P("wsum3", [1, 3 * DM], BF16)
    owT = P("owT", [DM, H], BF16)
    ob = P("ob", [1, H])
    ff1w3 = P("ff1w3", [32, 128, H], BF16)
    ff1b = P("ff1b", [32, 128])
    ff2wT = P("ff2wT", [I_, H], BF16)
    ffb2 = P("ffb2", [1, H])
    y = nc.declare_dram_parameter("y", [SM, H], F32, isOutput=True)

    with tile.TileContext(nc) as tc:
        cst = tc.alloc_tile_pool(name="cst", bufs=1)
        dram = tc.alloc_tile_pool(name="dram", bufs=1, space="DRAM")
        setp = tc.alloc_tile_pool(name="setp", bufs=1)
        ps_set = tc.alloc_tile_pool(name="ps_set", bufs=2, space="PSUM")

        rs_in = dram.tile([S, H], BF16)
        rs_out = dram.tile([SM, H], BF16)

        ones_f = cst.tile([1, 128], F32)
        nc.gpsimd.memset(ones_f, 1.0)
        ones_b = cst.tile([1, 128], BF16)
        nc.vector.tensor_copy(ones_b[:], ones_f[:])

        def load_vec(p):
            t = setp.tile([1, H], F32, tag=f"v_{p.name}")
            nc.sync.dma_start(out=t[:], in_=p[:])
            return t

        ob_v, ffb2_v = load_vec(ob), load_vec(ffb2)

        def bcast(v, name):
            bc = cst.tile([128, H], F32, tag=f"bc_{name}", name=f"bc_{name}")
            for hf in range(H // 512):
                ps = ps_set.tile([128, 512], F32)
                nc.tensor.matmul(ps[:], ones_f[0:1, :], v[0:1, hf * 512:(hf + 1) * 512],
                                 start=True, stop=True)
                nc.vector.tensor_copy(bc[:, hf * 512:(hf + 1) * 512], ps[:])
            return bc

        ob_bc = bcast(ob_v, "ob")
        ffb2_bc = bcast(ffb2_v, "ffb2")

        bqkv_sb = setp.tile([1, 3 * DM], F32)
        nc.sync.dma_start(out=bqkv_sb[:], in_=bqkv[:])
        qkvb_pp = []
        for j in range(3):
            ps = ps_set.tile([128, 512], F32)
            nc.tensor.matmul(ps[:, 0:1],
                             bqkv_sb[0:1, j * 128:(j + 1) * 128],
                             ones_f[0:1, 0:1], start=True, stop=True)
            t = cst.tile([128, 1], F32, tag=f"b_pp{j}")
            nc.vector.tensor_copy(t[:], ps[:, 0:1])
            qkvb_pp.append(t)

        ident_b = cst.tile([128, 128], BF16)
        make_identity(nc, ident_b)
        ident_f = cst.tile([32, 32], F32)
        make_identity(nc, ident_f)
        ffb1_ld = setp.tile([32, 128], F32)
        nc.sync.dma_start(out=ffb1_ld[:], in_=ff1b[:])
        ps = ps_set.tile([128, 512], F32)
        nc.tensor.transpose(ps[:, 0:32], ffb1_ld[:], ident_f[:])
        ffb1_pp = cst.tile([128, 32], F32)
        nc.vector.tensor_copy(ffb1_pp[:], ps[:, 0:32])

        wq_sb = []
        for hc in range(HC):
            t = cst.tile([128, 3 * DM], BF16, tag=f"wqkv{hc}")
            nc.sync.dma_start(out=t[:], in_=wqkvT[hc * 128:(hc + 1) * 128, :])
            wq_sb.append(t)
        owT_sb = cst.tile([DM, H], BF16, tag="owT")
        nc.sync.dma_start(out=owT_sb[:], in_=owT[:])
        ones_cb = cst.tile([128, 1], BF16)
        nc.gpsimd.memset(ones_cb, 1.0)
        ws_sb = cst.tile([1, 3 * DM], BF16, tag="ws")
        nc.sync.dma_start(out=ws_sb[:], in_=wsum3[:])

        ps_set.release()
        setp.release()

        ff = tc.alloc_tile_pool(name="ff", bufs=1)
        attnp = tc.alloc_tile_pool(name="attnp", bufs=1)
        xntp = tc.alloc_tile_pool(name="xntp", bufs=1)

        xnT = xntp.tile([128, HC, S], BF16, name="xT")
        for t in range(NT):
            nc.sync.dma_start_transpose(
                out=xnT[:, :, t * 128:(t + 1) * 128],
                in_=x_bf[t * 128:(t + 1) * 128, :])

        QTp = attnp.tile([128, S], BF16, name="QTp")
        KTp = attnp.tile([128, S], BF16, name="KTp")
        vb = attnp.tile([128, NT, 130], BF16, name="vb")
        nc.gpsimd.memset(vb[:, :, 64:65], 1.0)
        nc.gpsimd.memset(vb[:, :, 129:130], 1.0)

        with tc.tile_pool(name="ps_qkv", bufs=2, space="PSUM") as ps_qkv, \
             tc.tile_pool(name="vt_sb", bufs=3) as vt_sb:
            def v_trans(r, vt):
                for tb in range(4):
                    ti = r * 4 + tb
                    vtp = ps_qkv.tile([128, 128], BF16, tag="vtp")
                    nc.tensor.transpose(vtp[:], vt[:, tb * 128:(tb + 1) * 128],
                                        ident_b[:])
                    for h in range(2):
                        nc.vector.tensor_copy(vb[:, ti, h * 65:h * 65 + 64],
                                              vtp[:, h * 64:(h + 1) * 64])

            vt_prev = None
            for r in range(ST):
                sl = slice(r * 512, (r + 1) * 512)
                vps = ps_qkv.tile([128, 512], F32, tag="v")
                for hc in range(HC):
                    nc.tensor.matmul(vps[0:1, :], ones_cb[:, :],
                                     xnT[:, hc, sl],
                                     start=(hc == 0), stop=(hc == 7))
                mu = vt_sb.tile([1, 512], BF16, tag="mu")
                nc.vector.tensor_scalar_mul(mu[:], vps[0:1, :], 1.0 / H)
                qk = ps_qkv.tile([128, 1024], F32, tag="qk")
                for j, dest in ((0, QTp), (1, KTp)):
                    for hc in range(HC):
                        nc.tensor.matmul(qk[:, j * 512:(j + 1) * 512],
                                         wq_sb[hc][:, j * 128:(j + 1) * 128],
                                         xnT[:, hc, sl],
                                         start=(hc == 0), stop=False)
                    nc.tensor.matmul(qk[:, j * 512:(j + 1) * 512],
                                     ws_sb[0:1, j * 128:(j + 1) * 128],
                                     mu[0:1, :], start=False, stop=True,
                                     skip_group_check=True)
                    nc.vector.tensor_scalar_add(
                        dest[:, sl], qk[:, j * 512:(j + 1) * 512], qkvb_pp[j][:])
                if vt_prev is not None:
                    v_trans(r - 1, vt_prev)
                for hc in range(HC):
                    nc.tensor.matmul(vps[:],
                                     wq_sb[hc][:, 2 * 128:3 * 128],
                                     xnT[:, hc, sl],
                                     start=(hc == 0), stop=False)
                nc.tensor.matmul(vps[:], ws_sb[0:1, 2 * 128:3 * 128],
                                 mu[0:1, :], start=False, stop=True,
                                 skip_group_check=True)
                vt = vt_sb.tile([128, 512], BF16, tag="vt")
                nc.vector.tensor_scalar_add(vt[:], vps[:], qkvb_pp[2][:])
                vt_prev = vt
            v_trans(ST - 1, vt_prev)

        xntp.release()

        xn2T = ff.tile([128, HC, SM], BF16, name="xn2T")
        x2_t = []

        with tc.tile_pool(name="ps_sps", bufs=2, space="PSUM") as ps_sps, \
             tc.tile_pool(name="ps_ctx", bufs=2, space="PSUM") as ps_ctx, \
             tc.tile_pool(name="exp", bufs=3) as expp, \
             tc.tile_pool(name="ctxp", bufs=3) as ctxp, \
             tc.tile_pool(name="aop", bufs=2) as aop, \
             tc.tile_pool(name="rcp", bufs=8) as rcp, \
             tc.tile_pool(name="ffs", bufs=2) as ffs:

            def chunk_ln2(r):
                rl = ffs.tile([64, H], BF16, tag="rl")
                nc.sync.dma_start(out=rl[:], in_=rs_out[r * 64:(r + 1) * 64, :])
                rf = ffs.tile([64, H], F32, tag="rf")
                nc.vector.tensor_copy(rf[:], rl[:])
                xo = ffs.tile([64, H], F32, tag="xo")
                nc.sync.dma_start(out=xo[:], in_=x_own[r * 64:(r + 1) * 64, :])
                x2 = ff.tile([64, H], F32, tag=f"x2_{r}", name=f"x2_{r}")
                nc.vector.tensor_add(x2[:], rf[:], xo[:])
                nc.vector.tensor_add(x2[:], x2[:], ob_bc[0:64, :])
                x2_t.append(x2)
                ns = ffs.tile([64, 1], F32, tag="s2")
                nc.vector.reduce_sum(out=ns[:], in_=x2[:], axis=AXX, negate=True)
                nm = ffs.tile([64, 1], F32, tag="nm2")
                nc.vector.tensor_scalar_mul(nm[:], ns[:], 1.0 / H)
                xn2 = ffs.tile([64, H], BF16, tag="xn2")
                nc.vector.tensor_scalar_add(xn2[:], x2[:], nm[:])
                nc.sync.dma_start_transpose(
                    out=xn2T[:, :, r * 64:(r + 1) * 64], in_=xn2[:])

            cps_by_r = {}
            ex_q = {}

            ctxT_by_r = {}

            def boundary_a(r):
                cps = cps_by_r.pop(r)
                ctxT = ctxp.tile([128, 512], BF16, tag="ctxT", name=f"ctxT{r}")
                for h in range(2):
                    den = rcp.tile([1, 512], F32, tag="den")
                    nc.vector.tensor_copy(den[:], cps[h][64:65, :])
                    rc = rcp.tile([1, 512], F32, tag="rc")
                    nc.vector.reciprocal_approx_fast(rc[:], den[:])
                    rcb = rcp.tile([1, 512], BF16, tag="rcb")
                    nc.vector.tensor_copy(rcb[:], rc[:])
                    bps = ps_sps.tile([128, 1024], F32, tag="sps",
                                      name=f"bps{r}_{h}")
                    nc.tensor.matmul(bps[0:64, 0:512], ones_b[0:1, 0:64],
                                     rcb[0:1, :], start=True, stop=True)
                    bsb = rcp.tile([64, 512], F32, tag="bsb")
                    nc.vector.tensor_copy(bsb[:], bps[0:64, 0:512])
                    nc.vector.tensor_mul(ctxT[h * 64:(h + 1) * 64, :],
                                         cps[h][0:64, :], bsb[:])
                ctxT_by_r[r] = ctxT

            def boundary_b(r):
                ctxT = ctxT_by_r.pop(r)
                for j in range(4):
                    ops = ps_sps.tile([128, 1024], F32, tag="sps",
                                      name=f"ops{r}_{j}")
                    ao = aop.tile([128, H], BF16, tag="ao")
                    for hf in range(2):
                        nc.tensor.matmul(ops[:, hf * 512:(hf + 1) * 512],
                                         ctxT[:, j * 128:(j + 1) * 128],
                                         owT_sb[:, hf * 512:(hf + 1) * 512],
                                         start=True, stop=True)
                        nc.vector.tensor_copy(ao[:, hf * 512:(hf + 1) * 512],
                                              ops[:, hf * 512:(hf + 1) * 512])
                    nc.sync.dma_start(
                        out=rs_in[r * 512 + j * 128:r * 512 + (j + 1) * 128, :],
                        in_=ao[:])
                nc.gpsimd.collective_compute(
                    "ReduceScatter", ALU.add, replica_groups=[list(range(NC))],
                    ins=[rs_in[r * 512:(r + 1) * 512, :]],
                    outs=[rs_out[r * 64:(r + 1) * 64, :]])
                chunk_ln2(r)

            NG = ST * NT
            for g in range(NG + 1):
                if g < NG:
                    r, t = divmod(g, NT)
                    if t == 0:
                        cps_by_r[r] = [ps_ctx.tile([65, 512], F32, tag=f"cps{h}",
                                                   name=f"cps{r}_{h}")
                                       for h in range(2)]
                    sl = slice(r * 512, (r + 1) * 512)
                    sps = ps_sps.tile([128, 1024], F32, tag="sps")
                    for h in range(2):
                        hs = slice(h * 64, (h + 1) * 64)
                        nc.tensor.matmul(sps[:, h * 512:(h + 1) * 512],
                                         KTp[hs, t * 128:(t + 1) * 128],
                                         QTp[hs, sl], start=True, stop=True)
                    ex = expp.tile([128, 1024], BF16, tag="ex")
                    nc.scalar.activation(ex[:], sps[:], AF.Exp, scale=0.125)
                    ex_q[g] = ex
                if g >= 1:
                    gp = g - 1
                    rp, tp = divmod(gp, NT)
                    for h in range(2):
                        nc.tensor.matmul(cps_by_r[rp][h][:],
                                         vb[:, tp, h * 65:(h + 1) * 65],
                                         ex_q[gp][:, h * 512:(h + 1) * 512],
                                         start=(tp == 0), stop=(tp == NT - 1),
                                         skip_group_check=True)
                    del ex_q[gp]
                if g >= NT + 2 and (g - 2) % NT == 0:
                    boundary_a((g - 2) // NT - 1)
                if g >= NT + 5 and (g - 5) % NT == 0:
                    boundary_b((g - 5) // NT - 1)
            boundary_a(ST - 1)
            boundary_b(ST - 1)

        attnp.release()

        ffl = tc.alloc_tile_pool(name="ffl", bufs=1)
        with tc.tile_pool(name="w1p", bufs=8) as w1p, \
             tc.tile_pool(name="w2p", bufs=8) as w2p, \
             tc.tile_pool(name="yp", bufs=3) as yp, \
             tc.tile_pool(name="ps_f1", bufs=2, space="PSUM") as ps_f1, \
             tc.tile_pool(name="ps_f2", bufs=1, space="PSUM") as ps_f2:

            hT = [ffl.tile([128, SM], BF16, tag=f"hT{i}", name=f"hT{i}")
                  for i in range(32)]
            for it in range(32):
                w1t = w1p.tile([128, HC, 128], BF16, tag="w1")
                nc.sync.dma_start(out=w1t[:], in_=ff1w3[it, :, :])
                ps1 = ps_f1.tile([128, 512], F32, tag="f1")
                for hc in range(HC):
                    nc.tensor.matmul(ps1[:], w1t[:, hc, :], xn2T[:, hc, :],
                                     start=(hc == 0), stop=(hc == 7))
                nc.scalar.activation(hT[it][:], ps1[:], AF.Relu,
                                     bias=ffb1_pp[:, it:it + 1])

            for hf in range(2):
                sl = slice(hf * 512, (hf + 1) * 512)
                yps = [ps_f2.tile([128, 512], F32, name=f"yps{hf}_{i}",
                                  tag=f"yps{i}", bufs=1) for i in range(4)]
                for ic in range(32):
                    w2t = w2p.tile([128, 512], BF16, tag="w2")
                    nc.sync.dma_start(
                        out=w2t[:],
                        in_=ff2wT[ic * 128:(ic + 1) * 128,
                                  hf * 512:(hf + 1) * 512])
                    for s4 in range(4):
                        nc.tensor.matmul(yps[s4][:],
                                         hT[ic][:, s4 * 128:(s4 + 1) * 128],
                                         w2t[:], start=(ic == 0), stop=(ic == 31),
                                         skip_group_check=True)
                for s4 in range(4):
                    for half in range(2):
                        r = s4 * 2 + half
                        hsl = slice(half * 64, (half + 1) * 64)
                        yt = yp.tile([64, 512], F32, tag="yt")
                        nc.vector.tensor_add(yt[:], yps[s4][hsl, :],
                                             ffb2_bc[0:64, sl])
                        nc.vector.tensor_add(yt[:], yt[:], x2_t[r][:, sl])
                        nc.sync.dma_start(
                            out=y[s4 * 128 + half * 64:s4 * 128 + (half + 1) * 64,
                                  sl],
                            in_=yt[:])

        ffl.release()
        ff.release()
        dram.release()
        cst.release()

    nc.compile()
    return nc


def make_in_maps(inputs):
    f32 = lambda a: np.ascontiguousarray(np.asarray(a, dtype=np.float32))
    bf = lambda a: np.ascontiguousarray(np.asarray(a, dtype=np.float32)
                                        .astype(ml_dtypes.bfloat16))
    x = f32(inputs["x"])
    x_b = np.ascontiguousarray(x.astype(ml_dtypes.bfloat16))
    anw, anb = f32(inputs["an_w"]), f32(inputs["an_b"])
    fnw, fnb = f32(inputs["fn_w"]), f32(inputs["fn_b"])
    q_w, k_w, v_w = f32(inputs["q_w"]), f32(inputs["k_w"]), f32(inputs["v_w"])
    o_w = f32(inputs["o_w"])
    ff1_w, ff2_w = f32(inputs["ff1_w"]), f32(inputs["ff2_w"])

    w1_eff = ff1_w * fnw[None, :]
    b1_eff = f32(inputs["ff1_b"]) + ff1_w @ fnb
    ff1w3 = np.ascontiguousarray(
        w1_eff.reshape(32, 128, HC, 128).transpose(0, 3, 2, 1)
        .reshape(32, 128, H).astype(ml_dtypes.bfloat16))
    ff2wT = bf(ff2_w.T)
    ff1b = np.ascontiguousarray(b1_eff.reshape(32, 128))
    row = lambda a: np.ascontiguousarray(a.reshape(1, -1))

    in_maps = []
    for m in range(NC):
        dm = slice(m * DM, (m + 1) * DM)
        wq = (q_w[dm] * anw[None, :]).T
        wk = (k_w[dm] * anw[None, :]).T
        wv = (v_w[dm] * anw[None, :]).T
        wqkvT = np.ascontiguousarray(
            np.concatenate([wq, wk, wv], axis=1).astype(ml_dtypes.bfloat16))
        bq = f32(inputs["q_b"])[dm] + q_w[dm] @ anb
        bk = f32(inputs["k_b"])[dm] + k_w[dm] @ anb
        bv = f32(inputs["v_b"])[dm] + v_w[dm] @ anb
        wsum3v = -np.concatenate([wq, wk, wv], axis=1).sum(axis=0)
        in_maps.append({
            "x_bf": x_b,
            "wsum3": np.ascontiguousarray(
                wsum3v.reshape(1, -1).astype(ml_dtypes.bfloat16)),
            "x_own": np.ascontiguousarray(x[row_perm(m)]),
            "wqkvT": wqkvT,
            "bqkv": row(np.concatenate([bq, bk, bv])),
            "owT": bf(o_w[:, dm].T),
            "ob": row(f32(inputs["o_b"])),
            "ff1w3": ff1w3,
            "ff1b": ff1b,
            "ff2wT": ff2wT,
            "ffb2": row(f32(inputs["ff2_b"])),
        })
    return in_maps


def row_perm(m):
    return np.concatenate(
        [np.arange(r * 512 + m * 64, r * 512 + (m + 1) * 64) for r in range(ST)])


def kernel(**inputs) -> np.ndarray:
    from concourse.bass_utils import run_bass_kernel_spmd
    if "nc" not in _CACHE:
        _CACHE["nc"] = build_nc()
    nc = _CACHE["nc"]
    in_maps = make_in_maps(inputs)
    res = run_bass_kernel_spmd(nc, in_maps, core_ids=list(range(NC)))
    out = np.empty((S, H), dtype=np.float32)
    for m in range(NC):
        out[row_perm(m)] = res.results[m]["y"]
    return out
